# revision 4
# baseline (speedup 1.0000x reference)
"""AffinityLoss BCE kernel for 8 Trainium2 NeuronCores.

Computes mean BCE between prediction [4,4096,4096] (probabilities) and the
pairwise label-equality affinity derived from target [4,512,512]:

    aff[b,i,j] = (lab[b,i] == lab[b,j]),  lab = target[:, ::8, ::8].flatten
    loss = mean( -(aff*log(p) + (1-aff)*log(1-p)) )

Key identity used on-chip (one transcendental per element):
    -loss_elem = log(q),  q = p if aff else (1-p)
    q = 0.5 + sign*(p-0.5),  sign = 2*aff-1
Per [128 x 4096] tile:
    aff_s = (labj == labi) - 0.5          # DVE tensor_scalar, bf16 4x mode
    t     = (p - 0.5) * aff_s             # DVE scalar_tensor_tensor, fp32
    acc  += sum(Ln(2*t + 0.5))            # ScalarE ACT with accum_out

Sharding: data-parallel over rows of the [B*4096, 4096] prediction;
core c handles batch c//2, row half c%2 (2048 rows = 16 row-blocks).
Each core returns per-(partition, block) partial sums [128,16]; the host
sums them in float64 and divides by the element count.
"""

import numpy as np
from ml_dtypes import bfloat16

import concourse.bacc as bacc
import concourse.tile as tile
import concourse.mybir as mybir
from concourse import bass_utils

B = 4
N = 4096            # (512//8)**2
STRIDE = 8
NUM_CLASSES = 182
IGNORE = 255
N_CORES = 8
ROWS_PER_CORE = (B * N) // N_CORES   # 2048
P = 128
BLOCKS = ROWS_PER_CORE // P          # 16
F = N                                # free dim of a tile

_cache = {}
last_results = None  # test harness reads exec_time_ns off this


def _build():
    if "nc" in _cache:
        return _cache["nc"]

    f32 = mybir.dt.float32
    bf16 = mybir.dt.bfloat16
    Alu = mybir.AluOpType
    Act = mybir.ActivationFunctionType

    nc = bacc.Bacc("TRN2", target_bir_lowering=False, debug=False)
    pred = nc.dram_tensor("pred", [ROWS_PER_CORE, F], f32, kind="ExternalInput").ap()
    labj = nc.dram_tensor("labj", [P, F], bf16, kind="ExternalInput").ap()
    labi = nc.dram_tensor("labi", [P, BLOCKS], f32, kind="ExternalInput").ap()
    acc = nc.dram_tensor("acc", [P, BLOCKS], f32, kind="ExternalOutput").ap()

    with tile.TileContext(nc) as tc:
        with (
            tc.tile_pool(name="const", bufs=1) as cpool,
            tc.tile_pool(name="pin", bufs=3) as ppool,
            tc.tile_pool(name="aff", bufs=2) as apool,
            tc.tile_pool(name="tmul", bufs=2) as tpool,
            tc.tile_pool(name="lnout", bufs=2) as lpool,
        ):
            labj_sb = cpool.tile([P, F], bf16, tag="labj")
            nc.sync.dma_start(labj_sb[:], labj[:])
            labi_sb = cpool.tile([P, BLOCKS], f32, tag="labi")
            nc.sync.dma_start(labi_sb[:], labi[:])
            acc_sb = cpool.tile([P, BLOCKS], f32, tag="acc")
            half = cpool.tile([P, 1], f32, tag="half")
            nc.gpsimd.memset(half[:], 0.5)

            for t in range(BLOCKS):
                p_t = ppool.tile([P, F], f32, tag="p")
                nc.sync.dma_start(p_t[:], pred[t * P:(t + 1) * P, :])

                aff = apool.tile([P, F], bf16, tag="aff")
                nc.vector.tensor_scalar(
                    aff[:], labj_sb[:], labi_sb[:, t:t + 1], 0.5,
                    Alu.is_equal, Alu.subtract,
                )
                tm = tpool.tile([P, F], f32, tag="t")
                nc.vector.scalar_tensor_tensor(
                    tm[:], p_t[:], 0.5, aff[:], Alu.subtract, Alu.mult,
                )
                ln = lpool.tile([P, F], f32, tag="ln")
                nc.scalar.activation(
                    ln[:], tm[:], Act.Ln, bias=half[:], scale=2.0,
                    accum_out=acc_sb[:, t:t + 1],
                )

            nc.sync.dma_start(acc[:], acc_sb[:])

    nc.compile()
    _cache["nc"] = nc
    return nc


def make_in_maps(prediction, target):
    prediction = np.asarray(prediction, dtype=np.float32)
    target = np.asarray(target)
    lab = target[:, ::STRIDE, ::STRIDE]
    lab = np.where(lab == IGNORE, NUM_CLASSES, lab)
    flat = lab.reshape(B, N).astype(np.float32).astype(bfloat16)

    in_maps = []
    per_batch = N_CORES // B
    for c in range(N_CORES):
        b = c // per_batch
        r0 = (c % per_batch) * ROWS_PER_CORE
        in_maps.append({
            "pred": np.ascontiguousarray(prediction[b, r0:r0 + ROWS_PER_CORE, :]),
            "labj": np.ascontiguousarray(np.broadcast_to(flat[b], (P, N))),
            "labi": np.ascontiguousarray(
                flat[b, r0:r0 + ROWS_PER_CORE].reshape(BLOCKS, P).T.astype(np.float32)
            ),
        })
    return in_maps


def kernel(prediction, target):
    global last_results
    nc = _build()
    in_maps = make_in_maps(prediction, target)
    res = bass_utils.run_bass_kernel_spmd(nc, in_maps, core_ids=list(range(N_CORES)))
    last_results = res
    total = 0.0
    for r in res.results:
        total += r["acc"].astype(np.float64).sum()
    loss = -total / float(B * N * N)
    return np.float32(loss)
